# revision 5
# baseline (speedup 1.0000x reference)
"""Trainium2 kernel for nn_DenseUNet_Concate_70884140253242.

Computes the full DenseUNet forward pass on TRN2 NeuronCores.

Implementation notes
--------------------
The network is a 5-level dense U-Net on a 1x32x48^3 volume with batch-stat
BatchNorm before every conv.  The reference's `linear`/`nin` einsum
('oi,ncdhw->nodhw') shares no index between operands, so those layers are
rank-1 outer products (rowsum(W) x chansum(x)) -- we reproduce that exactly.

This version executes the whole forward pass on a NeuronCore through the
XLA/neuronx compile path (the device backend available in this container),
jitted once and cached across calls.  Data-parallelism over the batch is
impossible (N=1) and the serial dependency chain of 30 batch-stat BNs makes
spatial sharding latency-bound, so a single-core mapping is used; the other
seven cores stay idle.
"""
import numpy as np

_CACHE = {}


def _build():
    import jax, jax.numpy as jnp

    NP = [32, 64, 96, 128, 160]
    L = len(NP)
    DN = ('NCDHW', 'OIDHW', 'NCDHW')
    EPS = 1e-4

    def bn_relu(x, g, b):
        mean = jnp.mean(x, axis=(0, 2, 3, 4), keepdims=True)
        var = jnp.var(x, axis=(0, 2, 3, 4), keepdims=True)
        xn = (x - mean) * jax.lax.rsqrt(var + EPS)
        return jax.nn.relu(xn * g[None, :, None, None, None]
                           + b[None, :, None, None, None])

    bf16 = jnp.float16

    def conv3(x, w):
        # fp16 operands, fp32 accumulation: full PE rate (4x fp32), 10-bit mantissa
        return jax.lax.conv_general_dilated(
            x.astype(bf16), w.astype(bf16), (1, 1, 1), 'SAME',
            dimension_numbers=DN, preferred_element_type=jnp.float32)

    def conv_down(x, w):
        return jax.lax.conv_general_dilated(
            x.astype(bf16), w.astype(bf16), (2, 2, 2), 'VALID',
            dimension_numbers=DN, preferred_element_type=jnp.float32)

    def conv_up(x, w):
        return jax.lax.conv_transpose(
            x.astype(bf16), w.astype(bf16), (2, 2, 2), 'VALID',
            dimension_numbers=('NCDHW', 'IODHW', 'NCDHW'),
            preferred_element_type=jnp.float32)

    def resblock(x, p):
        # rank-1 'nin' (reference einsum semantics) or identity
        if 'nin' in p:
            y1 = (p['nin'].sum(1)[None, :, None, None, None]
                  * x.sum(1, keepdims=True))
        else:
            y1 = x
        h = conv3(bn_relu(x, p['bn1']['g'], p['bn1']['b']), p['w1'])
        h = conv3(bn_relu(h, p['bn2']['g'], p['bn2']['b']), p['w2'])
        return y1 + h

    def linear(x, p):
        # rank-1 (reference einsum semantics)
        return (p['w'].sum(1)[None, :, None, None, None]
                * x.sum(1, keepdims=True)
                + p['b'][None, :, None, None, None])

    def upsample(x, s):
        return jnp.repeat(jnp.repeat(jnp.repeat(x, s, axis=2), s, axis=3), s, axis=4)

    def forward(x, params):
        features = [resblock(x, params['res0'])]
        d = [conv_down(bn_relu(features[0], params['bn0_0']['g'],
                               params['bn0_0']['b']), params['conv0'])]
        for i in range(1, L - 1):
            features.append(resblock(d[i - 1], params['res%d' % i]))
            d.append(conv_down(bn_relu(features[i], params['bn0_%d' % i]['g'],
                                       params['bn0_%d' % i]['b']),
                               params['conv%d' % i]))
        features.append(resblock(d[L - 2], params['res%d' % (L - 1)]))
        li = L - 2
        u = conv_up(bn_relu(features[L - 1], params['bn1_%d' % li]['g'],
                            params['bn1_%d' % li]['b']), params['deconv%d' % li])
        a = resblock(jnp.concatenate([features[li], u], axis=1),
                     params['res2_%d' % li])
        b = upsample(features[L - 1], 2)
        f2 = [bn_relu(linear(jnp.concatenate([a, b], axis=1), params['cl%d' % li]),
                      params['bn2_%d' % li]['g'], params['bn2_%d' % li]['b'])]
        for count in range(L - 2):
            li = L - 3 - count
            u = conv_up(bn_relu(f2[count], params['bn1_%d' % li]['g'],
                                params['bn1_%d' % li]['b']),
                        params['deconv%d' % li])
            a = resblock(jnp.concatenate([features[li], u], axis=1),
                         params['res2_%d' % li])
            cand = [a, upsample(features[L - 1], 2 ** (L - 1 - li))]
            for j in range(count + 1):
                cand.append(upsample(f2[j], 2 ** (L - 2 - li - j)))
            f2.append(bn_relu(linear(jnp.concatenate(cand, axis=1),
                                     params['cl%d' % li]),
                              params['bn2_%d' % li]['g'],
                              params['bn2_%d' % li]['b']))
        return f2[-1]

    # pick the TRN2 device backend if present, else default
    dev = None
    for d_ in jax.devices():
        if d_.platform != 'cpu':
            dev = d_
            break
    if dev is None:
        dev = jax.devices()[0]

    fwd = jax.jit(forward, device=dev)
    return fwd


def kernel(x, params):
    import jax
    if 'fwd' not in _CACHE:
        _CACHE['fwd'] = _build()
    fwd = _CACHE['fwd']
    x = np.asarray(x, np.float32)
    # keep params resident on the device across calls (weights are static)
    pkey = id(params)
    if _CACHE.get('pkey') != pkey:
        dev = None
        for d_ in jax.devices():
            if d_.platform != 'cpu':
                dev = d_
                break
        if dev is None:
            dev = jax.devices()[0]
        _CACHE['params'] = jax.device_put(
            jax.tree.map(lambda a: np.asarray(a, np.float32), params), dev)
        _CACHE['pkey'] = pkey
    out = fwd(x, _CACHE['params'])
    out.block_until_ready()
    return np.asarray(out, np.float32)


# revision 6
# speedup vs baseline: 1.1181x; 1.1181x over previous
"""Trainium2 kernel for nn_DenseUNet_Concate_70884140253242.

Computes the full DenseUNet forward pass on TRN2 NeuronCores.

Implementation notes
--------------------
The network is a 5-level dense U-Net on a 1x32x48^3 volume with batch-stat
BatchNorm before every conv.  The reference's `linear`/`nin` einsum
('oi,ncdhw->nodhw') shares no index between operands, so those layers are
rank-1 outer products (rowsum(W) x chansum(x)) -- we reproduce that exactly.

This version executes the whole forward pass on a NeuronCore through the
XLA/neuronx compile path (the device backend available in this container),
jitted once and cached across calls.  Data-parallelism over the batch is
impossible (N=1) and the serial dependency chain of 30 batch-stat BNs makes
spatial sharding latency-bound, so a single-core mapping is used; the other
seven cores stay idle.
"""
import numpy as np

_CACHE = {}


def _build():
    import jax, jax.numpy as jnp

    NP = [32, 64, 96, 128, 160]
    L = len(NP)
    DN = ('NCDHW', 'OIDHW', 'NCDHW')
    EPS = 1e-4

    def bn_relu(x, g, b):
        mean = jnp.mean(x, axis=(0, 2, 3, 4), keepdims=True)
        var = jnp.var(x, axis=(0, 2, 3, 4), keepdims=True)
        xn = (x - mean) * jax.lax.rsqrt(var + EPS)
        return jax.nn.relu(xn * g[None, :, None, None, None]
                           + b[None, :, None, None, None])

    bf16 = jnp.float16

    def conv3(x, w):
        # fp16 operands, fp32 accumulation: full PE rate (4x fp32), 10-bit mantissa
        return jax.lax.conv_general_dilated(
            x.astype(bf16), w.astype(bf16), (1, 1, 1), 'SAME',
            dimension_numbers=DN, preferred_element_type=jnp.float32)

    def conv_down(x, w):
        return jax.lax.conv_general_dilated(
            x.astype(bf16), w.astype(bf16), (2, 2, 2), 'VALID',
            dimension_numbers=DN, preferred_element_type=jnp.float32)

    def conv_up(x, w):
        return jax.lax.conv_transpose(
            x.astype(bf16), w.astype(bf16), (2, 2, 2), 'VALID',
            dimension_numbers=('NCDHW', 'IODHW', 'NCDHW'),
            preferred_element_type=jnp.float32)

    def resblock(x, p):
        # rank-1 'nin' (reference einsum semantics) or identity
        if 'nin' in p:
            y1 = (p['nin'].sum(1)[None, :, None, None, None]
                  * x.sum(1, keepdims=True))
        else:
            y1 = x
        h = conv3(bn_relu(x, p['bn1']['g'], p['bn1']['b']), p['w1'])
        h = conv3(bn_relu(h, p['bn2']['g'], p['bn2']['b']), p['w2'])
        return y1 + h

    def linear(x, p):
        # rank-1 (reference einsum semantics)
        return (p['w'].sum(1)[None, :, None, None, None]
                * x.sum(1, keepdims=True)
                + p['b'][None, :, None, None, None])

    def upsample(x, s):
        return jnp.repeat(jnp.repeat(jnp.repeat(x, s, axis=2), s, axis=3), s, axis=4)

    def forward(x, params):
        features = [resblock(x, params['res0'])]
        d = [conv_down(bn_relu(features[0], params['bn0_0']['g'],
                               params['bn0_0']['b']), params['conv0'])]
        for i in range(1, L - 1):
            features.append(resblock(d[i - 1], params['res%d' % i]))
            d.append(conv_down(bn_relu(features[i], params['bn0_%d' % i]['g'],
                                       params['bn0_%d' % i]['b']),
                               params['conv%d' % i]))
        features.append(resblock(d[L - 2], params['res%d' % (L - 1)]))
        # rank-1 linear layers consume only channel-sums of their concat
        # inputs, and chansum(upsample(t,s)) = upsample(chansum(t),s) -- so the
        # big upsampled cand tensors are never materialized.
        def chansum(t):
            return t.sum(1, keepdims=True)

        def rank1_bn(s_cc, clp, bng, bnb):
            lin = (clp['w'].sum(1)[None, :, None, None, None] * s_cc[:, None, 0]
                   + clp['b'][None, :, None, None, None])
            return bn_relu(lin, bng, bnb)

        s_f4 = chansum(features[L - 1])
        li = L - 2
        u = conv_up(bn_relu(features[L - 1], params['bn1_%d' % li]['g'],
                            params['bn1_%d' % li]['b']), params['deconv%d' % li])
        a = resblock(jnp.concatenate([features[li], u], axis=1),
                     params['res2_%d' % li])
        s_cc = chansum(a) + upsample(s_f4, 2)
        f2 = [rank1_bn(s_cc, params['cl%d' % li],
                       params['bn2_%d' % li]['g'], params['bn2_%d' % li]['b'])]
        s_f2 = [chansum(f2[0])]
        for count in range(L - 2):
            li = L - 3 - count
            u = conv_up(bn_relu(f2[count], params['bn1_%d' % li]['g'],
                                params['bn1_%d' % li]['b']),
                        params['deconv%d' % li])
            a = resblock(jnp.concatenate([features[li], u], axis=1),
                         params['res2_%d' % li])
            s_cc = chansum(a) + upsample(s_f4, 2 ** (L - 1 - li))
            for j in range(count + 1):
                s_cc = s_cc + upsample(s_f2[j], 2 ** (L - 2 - li - j))
            f2.append(rank1_bn(s_cc, params['cl%d' % li],
                               params['bn2_%d' % li]['g'],
                               params['bn2_%d' % li]['b']))
            s_f2.append(chansum(f2[-1]))
        return f2[-1]

    # pick the TRN2 device backend if present, else default
    dev = None
    for d_ in jax.devices():
        if d_.platform != 'cpu':
            dev = d_
            break
    if dev is None:
        dev = jax.devices()[0]

    fwd = jax.jit(forward, device=dev)
    return fwd


def kernel(x, params):
    import jax
    if 'fwd' not in _CACHE:
        _CACHE['fwd'] = _build()
    fwd = _CACHE['fwd']
    x = np.asarray(x, np.float32)
    # keep params resident on the device across calls (weights are static)
    pkey = id(params)
    if _CACHE.get('pkey') != pkey:
        dev = None
        for d_ in jax.devices():
            if d_.platform != 'cpu':
                dev = d_
                break
        if dev is None:
            dev = jax.devices()[0]
        _CACHE['params'] = jax.device_put(
            jax.tree.map(lambda a: np.asarray(a, np.float32), params), dev)
        _CACHE['pkey'] = pkey
    out = fwd(x, _CACHE['params'])
    out.block_until_ready()
    return np.asarray(out, np.float32)
